# revision 17
# baseline (speedup 1.0000x reference)
"""Trainium2 Bass kernel for a GPT-2 transformer layer (B=4, T=2048, C=1024, H=16).

Sharding: 8 cores, one batch per core-pair; each core owns 1024 query tokens
(two 512-row blocks: an "early" block qbA and a "late" block qbB chosen so the
per-core causal attention work is balanced and the SPMD program is uniform).
No collectives: each core computes K/V for all 2048 tokens of its batch
(small redundancy), attention + MLP for its own rows only.

Causal structure (uniform across cores, causality in host-built mask data):
  qbA (query cols 0:512 of xq)  -> k-tiles 0..7,  additive mask on all 8
  qbB (query cols 512:1024)     -> k-tiles 0..15, additive mask on tiles 8..15
Host assignment (batch b = core//2, j = core%2):
  qbA = rows [j*512 : (j+1)*512),  qbB = rows [1024+j*512 : 1536+j*512)
qbA's needed keys are within [0:1024); qbB's keys [0:1024) are always fully
allowed (unmasked tiles) and keys [1024:2048) carry the mask. k-tiles beyond
the causal frontier are skipped entirely.

LayerNorms are folded into the matmuls: the device scales x rows by
rsqrt(var+eps) per token and two augmented contraction rows (mu*r, 1) paired
with host-folded weight rows (-colsum(g*W), ln_b@W + b) add the mean/bias
terms inside the same matmul accumulation.

Attention runs transposed (S^T[k,q] tiles): softmax denominators come from an
extra ones-column in the PV stationary; per-head 1/den is applied after PV
(gpsimd partition-broadcast + multiply). All matmuls are bf16 with f32 PSUM
accumulation.
"""

import numpy as np
import ml_dtypes

import concourse.bass as bass
import concourse.mybir as mybir
import concourse.tile as tile
from concourse import bacc
from concourse.bass import ts
from concourse.bass_utils import run_bass_kernel_spmd
from concourse.masks import make_identity

B, T, C, H = 4, 2048, 1024, 16
D = C // H          # 64
TQ = T // 2         # own query tokens per core = 1024
NCORES = 8
EPS = 1e-5
MASK_VAL = -1e30

F32 = mybir.dt.float32
BF16 = mybir.dt.bfloat16
AF = mybir.ActivationFunctionType

NT = T // 128        # 16 token tiles (all tokens)
NQ = TQ // 128       # 8 token tiles (own tokens)
NC8 = C // 128       # 8 c tiles
NF = 4 * C // 128    # 32 fc hidden tiles


def _ln_stats(nc, pool, x_t, ncols):
    """Per-partition mean/rsqrt stats of a [128, ncols] tile.
    Returns (r, m): r = rsqrt(var+eps), m = mu * r, both [128, 1] f32."""
    s = pool.tile([128, 1], F32, tag="ln_sum", name="ln_sum")
    ss = pool.tile([128, 1], F32, tag="ln_ssq", name="ln_ssq")
    trash = pool.tile([128, ncols], BF16, tag="ln_trash", name="ln_trash", bufs=1)
    nc.scalar.activation(trash[:], x_t[:], AF.Copy, accum_out=s[:])
    nc.scalar.activation(trash[:], x_t[:], AF.Square, accum_out=ss[:])
    mu = pool.tile([128, 1], F32, tag="ln_mu", name="ln_mu")
    nc.vector.tensor_scalar_mul(mu[:], s[:], 1.0 / ncols)
    ex2 = pool.tile([128, 1], F32, tag="ln_ex2", name="ln_ex2")
    nc.vector.tensor_scalar_mul(ex2[:], ss[:], 1.0 / ncols)
    var = pool.tile([128, 1], F32, tag="ln_var", name="ln_var")
    nc.vector.tensor_mul(var[:], mu[:], mu[:])
    nc.vector.tensor_sub(var[:], ex2[:], var[:])
    nc.vector.tensor_scalar_add(var[:], var[:], EPS)
    std = pool.tile([128, 1], F32, tag="ln_std", name="ln_std")
    nc.scalar.sqrt(std[:], var[:])
    r = pool.tile([128, 1], F32, tag="ln_r", name="ln_r")
    nc.vector.reciprocal(r[:], std[:])
    return mu, r


def _ln_transpose(nc, sp, psp, ident, src_tile, dstT, n_tiles, wk):
    """LayerNorm ((x-mu)*rsqrt) token-major [128, C] tiles into bf16 and
    PE-transpose into dstT (c-major, bf16), 4 token tiles per PSUM drain.
    src_tile: callable tt -> f32 AP."""
    for tt0 in range(0, n_tiles, 4):
        xs_ts = []
        for tt in range(tt0, tt0 + 4):
            x_t = src_tile(tt)
            mu, r = _ln_stats(nc, sp, x_t, C)
            xs_t = wk.tile([128, C], BF16, tag="xs_t", name="xs_t", bufs=5)
            nc.vector.tensor_scalar(xs_t[:], x_t[:], mu[:], r[:],
                                    mybir.AluOpType.subtract,
                                    mybir.AluOpType.mult)
            xs_ts.append(xs_t)
        for cc in range(NC8):
            pst4 = psp.tile([128, 4, 128], BF16, tag="tr", name="pst4")
            for i in range(4):
                nc.tensor.transpose(pst4[:, i, :],
                                    xs_ts[i][:, ts(cc, 128)], ident[:])
            nc.vector.tensor_copy(
                dstT[cc][:, tt0 * 128:(tt0 + 4) * 128], pst4[:])


def build_program(gelu_fn=None, loop_n=1, has_bias=False):
    nc = bacc.Bacc("TRN2", target_bir_lowering=False, debug=False)
    if gelu_fn is None:
        gelu_fn = AF.Gelu

    xb = nc.dram_tensor("xb", [T, C], F32, kind="ExternalInput")
    xq = nc.dram_tensor("xq", [TQ, C], F32, kind="ExternalInput")
    maskc = nc.dram_tensor("maskc", [T, 512], BF16, kind="ExternalInput")
    w1aug = nc.dram_tensor("w1aug", [C + 1, 3 * C], BF16, kind="ExternalInput")
    wpaug = nc.dram_tensor("wpaug", [C + 1, C], BF16, kind="ExternalInput")
    w2aug = nc.dram_tensor("w2aug", [C + 1, 4 * C], BF16, kind="ExternalInput")
    w3aug = nc.dram_tensor("w3aug", [4 * C + 1, C], BF16, kind="ExternalInput")
    out = nc.dram_tensor("out", [TQ, C], F32, kind="ExternalOutput")

    with tile.TileContext(nc) as tc:
        with (
            tc.tile_pool(name="glob", bufs=1) as pg,
            tc.tile_pool(name="stats", bufs=2) as sp,
            tc.tile_pool(name="psacc", bufs=4, space="PSUM") as psa,
            tc.tile_pool(name="pstr", bufs=4, space="PSUM") as psp,
        ):
            ident = pg.tile([128, 128], BF16, tag="ident", name="ident")
            make_identity(nc, ident[:])

            import contextlib
            loop_cm = tc.For_i(0, loop_n, 1) if loop_n > 1 else contextlib.nullcontext()
            with loop_cm, tc.tile_pool(name="p34", bufs=1) as p34:
                attnT = [p34.tile([128, TQ], BF16, tag=f"attnT{dt}",
                                  name=f"attnT{dt}") for dt in range(NC8)]

                with tc.tile_pool(name="att", bufs=1) as pa:
                    V_sb = [[pa.tile([128, 8, 65], BF16, tag=f"V{tt}_{hb}",
                                     name=f"V{tt}_{hb}") for hb in range(2)]
                            for tt in range(NT)]
                    mask_sb = [pa.tile([128, 512], BF16, tag=f"mask{kt}",
                                       name=f"mask{kt}") for kt in range(NT)]
                    for kt in range(NT):
                        nc.sync.dma_start(mask_sb[kt][:], maskc[ts(kt, 128), :])

                    # ---------- Phase 1: LN1 + transpose (xb and xq) ----------
                    with tc.tile_pool(name="ph12", bufs=1) as p12, \
                         tc.tile_pool(name="w12", bufs=3) as wp, \
                         tc.tile_pool(name="wk12", bufs=2) as wk:
                        xsT = [p12.tile([128, T], BF16, tag=f"xsT{cc}",
                                        name=f"xsT{cc}") for cc in range(NC8)]
                        xqsT = [p12.tile([128, TQ], BF16, tag=f"xqsT{cc}",
                                         name=f"xqsT{cc}") for cc in range(NC8)]
                        ones_t = None
                        if has_bias:
                            ones_t = p12.tile([1, T], BF16, tag="ones_t",
                                              name="ones_t")
                            nc.vector.memset(ones_t[:, :], 1.0)

                        def _load_xb(tt):
                            t = wk.tile([128, C], F32, tag="xb_t", name="xb_t")
                            nc.sync.dma_start(t[:], xb[ts(tt, 128), :])
                            return t

                        def _load_xq(tt):
                            t = wk.tile([128, C], F32, tag="xb_t", name="xq_t")
                            nc.sync.dma_start(t[:], xq[ts(tt, 128), :])
                            return t

                        _ln_transpose(nc, sp, psp, ident, _load_xb,
                                      xsT, NT, wk)
                        _ln_transpose(nc, sp, psp, ident, _load_xq,
                                      xqsT, NQ, wk)

                        # ---------- Phase 2+3: QKV + attention, interleaved ----
                        def qkv_chain(dst, dst_slice, w_col0, n_blk,
                                      blk_src):
                            """One output column-block chain: 8 c-tiles (+ bias).
                            All 8 stationary tiles come in one strided DMA."""
                            w_t = wp.tile([128, NC8, 128], BF16, tag="w1_t",
                                          name="w1_t", bufs=3)
                            nc.sync.dma_start(
                                w_t[:],
                                w1aug[0:C, w_col0:w_col0 + 128].rearrange(
                                    "(cc p) n -> p cc n", p=128))
                            w_aug = None
                            if has_bias:
                                w_aug = wp.tile([1, 128], BF16, tag="w1_aug",
                                                name="w1_aug")
                                nc.sync.dma_start(
                                    w_aug[:], w1aug[C:C + 1, w_col0:w_col0 + 128])
                            for blk in range(n_blk):
                                ps = psa.tile([128, 512], F32, tag="acc",
                                              name="ps_qkv")
                                for cc in range(NC8):
                                    nc.tensor.matmul(ps[:], w_t[:, cc, :],
                                                     blk_src(cc, blk),
                                                     start=(cc == 0),
                                                     stop=(cc == NC8 - 1
                                                           and not has_bias))
                                if has_bias:
                                    nc.tensor.matmul(ps[:], w_aug[:],
                                                     ones_t[:, ts(blk, 512)],
                                                     start=False, stop=True)
                                nc.vector.tensor_copy(dst_slice(dst, blk), ps[:])

                        # V for all heads (token-major), per hd-block
                        def v_block(hb):
                            w_ts = []
                            for cc in range(NC8):
                                w_t = wp.tile([128, 512], BF16, tag="w1v_t",
                                              name="w1v_t", bufs=8)
                                nc.sync.dma_start(
                                    w_t[:],
                                    w1aug[ts(cc, 128),
                                          2 * C + hb * 512:2 * C + (hb + 1) * 512])
                                w_ts.append(w_t)
                            w_aug = None
                            if has_bias:
                                w_aug = wp.tile([1, 512], BF16, tag="w1v_aug",
                                                name="w1v_aug", bufs=2)
                                nc.sync.dma_start(
                                    w_aug[:],
                                    w1aug[C:C + 1,
                                          2 * C + hb * 512:2 * C + (hb + 1) * 512])
                            for tt in range(NT):
                                ps = psa.tile([128, 512], F32, tag="acc",
                                              name="ps_v")
                                for cc in range(NC8):
                                    nc.tensor.matmul(ps[:],
                                                     xsT[cc][:, ts(tt, 128)],
                                                     w_ts[cc][:],
                                                     start=(cc == 0),
                                                     stop=(cc == NC8 - 1
                                                           and not has_bias))
                                if has_bias:
                                    nc.tensor.matmul(ps[:],
                                                     ones_t[:, ts(tt, 128)],
                                                     w_aug[:],
                                                     start=False, stop=True)
                                vt = V_sb[tt][hb]
                                nc.vector.tensor_copy(
                                    vt[:, :, 0:64],
                                    ps[:].rearrange("p (h d) -> p h d", h=8))
                                nc.vector.memset(vt[:, :, 64:65], 1.0)

                        def attention_head(h, kt_t, qt_t, wk3):
                            ro = (h % 2) * 64
                            dt = h // 2
                            for qb, nkt in ((0, 8), (1, NT)):
                                psO = psa.tile([65, 512], F32, tag="acc",
                                               name="ps_O")
                                for g0 in range(0, nkt, 4):
                                    exps = []
                                    for kt in range(g0, g0 + 4):
                                        psS = psp.tile([128, 512], F32, tag="tr",
                                                       name="ps_S")
                                        nc.tensor.matmul(
                                            psS[:],
                                            kt_t[ro:ro + 64, ts(kt, 128)],
                                            qt_t[ro:ro + 64, ts(qb, 512)],
                                            start=True, stop=True)
                                        expP = wk3.tile([128, 512], BF16,
                                                        tag="expP", name="expP")
                                        nc.scalar.activation(expP[:], psS[:],
                                                             AF.Exp, scale=0.125)
                                        if qb == 0 or kt >= 8:
                                            nc.vector.tensor_mul(expP[:], expP[:],
                                                                 mask_sb[kt][:])
                                        exps.append(expP)
                                    for i, kt in enumerate(range(g0, g0 + 4)):
                                        nc.tensor.matmul(
                                            psO[:], V_sb[kt][h // 8][:, h % 8, :],
                                            exps[i][:],
                                            start=(kt == 0), stop=(kt == nkt - 1))
                                rcp = wk3.tile([1, 512], F32, tag="rcp",
                                               name="rcp")
                                nc.vector.reciprocal(rcp[:], psO[64:65, :])
                                rep = wk3.tile([64, 512], F32, tag="rep",
                                               name="rep", bufs=2)
                                nc.gpsimd.partition_broadcast(rep[:], rcp[:],
                                                              channels=64)
                                nc.vector.tensor_mul(
                                    attnT[dt][ro:ro + 64, ts(qb, 512)],
                                    psO[0:64, :], rep[:])

                        with tc.tile_pool(name="wk3", bufs=4) as wk3:
                            for half in range(2):
                                v_block(half)
                                for dt in range(half * 4, (half + 1) * 4):
                                    kt_t = pa.tile([128, T], BF16, tag="KT",
                                                   name="KT", bufs=2)
                                    qt_t = pa.tile([128, TQ], BF16, tag="QT",
                                                   name="QT", bufs=2)
                                    qkv_chain(
                                        kt_t, lambda d, b: d[:, ts(b, 512)],
                                        C + dt * 128, T // 512,
                                        lambda cc, b: xsT[cc][:, ts(b, 512)])
                                    qkv_chain(
                                        qt_t, lambda d, b: d[:, ts(b, 512)],
                                        dt * 128, TQ // 512,
                                        lambda cc, b: xqsT[cc][:, ts(b, 512)])
                                    attention_head(2 * dt, kt_t, qt_t, wk3)
                                    attention_head(2 * dt + 1, kt_t, qt_t, wk3)

                # ---------- Phase 4: proj + residual (att pool freed) ----------
                with tc.tile_pool(name="px2", bufs=1) as px2:
                    x2_sb = [px2.tile([128, C], F32, tag=f"x2_{qt}",
                                      name=f"x2_{qt}") for qt in range(NQ)]
                    with tc.tile_pool(name="w4", bufs=3) as wp4, \
                         tc.tile_pool(name="wk4", bufs=2) as wk4:
                        ones_row = None
                        if has_bias:
                            ones_row = px2.tile([1, TQ], BF16, tag="ones_row",
                                                name="ones_row")
                            nc.vector.memset(ones_row[:], 1.0)
                        for cb in range(2):
                            w_ts = []
                            for ht in range(NC8):
                                w_t = wp4.tile([128, 512], BF16, tag="wp_t",
                                               name="wp_t", bufs=9)
                                nc.sync.dma_start(w_t[:],
                                                  wpaug[ts(ht, 128), ts(cb, 512)])
                                w_ts.append(w_t)
                            if has_bias:
                                w_aug = wp4.tile([1, 512], BF16, tag="wp_aug",
                                                 name="wp_aug")
                                nc.sync.dma_start(w_aug[:],
                                                  wpaug[C:C + 1, ts(cb, 512)])
                            for qt in range(NQ):
                                ps = psa.tile([128, 512], F32, tag="acc",
                                              name="ps_p")
                                for ht in range(NC8):
                                    nc.tensor.matmul(
                                        ps[:], attnT[ht][:, ts(qt, 128)],
                                        w_ts[ht][:],
                                        start=(ht == 0),
                                        stop=(ht == NC8 - 1 and not has_bias))
                                if has_bias:
                                    nc.tensor.matmul(ps[:],
                                                     ones_row[:, ts(qt, 128)],
                                                     w_aug[:],
                                                     start=False, stop=True)
                                xq_t = wk4.tile([128, 512], F32, tag="xq_t",
                                                name="xq_t")
                                nc.sync.dma_start(xq_t[:],
                                                  xq[ts(qt, 128), ts(cb, 512)])
                                nc.vector.tensor_add(x2_sb[qt][:, ts(cb, 512)],
                                                     ps[:], xq_t[:])

                    _mlp(nc, tc, sp, psa, psp, ident, x2_sb, w2aug, w3aug, out,
                         gelu_fn, has_bias)

    nc.compile()
    return nc


def _mlp(nc, tc, sp, psa, psp, ident, x2_sb, w2aug, w3aug, out, gelu_fn,
         has_bias):
    # ---------- Phase 5: LN2 + transpose; 6: fc1+gelu; 7: fc2+residual ------
    with tc.tile_pool(name="pgel", bufs=1) as pgel, \
         tc.tile_pool(name="w7", bufs=3) as wp7:
        geluT = [pgel.tile([128, TQ], BF16, tag=f"geluT{ft}", name=f"geluT{ft}")
                 for ft in range(NF)]
        ones_b16 = None
        if has_bias:
            ones_b16 = pgel.tile([1, TQ], BF16, tag="ones_b16", name="ones_b16")
            nc.vector.memset(ones_b16[:], 1.0)
        w3_ts = {}
        for cb in range(2):
            for ft in range(NF):
                w_t = wp7.tile([128, 512], BF16, tag="w3_t", name="w3_t",
                               bufs=NF + 2)
                nc.sync.dma_start(w_t[:], w3aug[ts(ft, 128), ts(cb, 512)])
                w3_ts[(cb, ft)] = w_t
            if has_bias:
                w_aug = wp7.tile([1, 512], BF16, tag="w3_aug", name="w3_aug")
                nc.sync.dma_start(w_aug[:], w3aug[4 * C:4 * C + 1, ts(cb, 512)])
                w3_ts[(cb, "aug")] = w_aug

        with tc.tile_pool(name="ph56", bufs=1) as p56, \
             tc.tile_pool(name="w6", bufs=3) as wp6, \
             tc.tile_pool(name="wk5", bufs=2) as wk5:
            xs2T = [p56.tile([128, TQ], BF16, tag=f"xs2T{cc}", name=f"xs2T{cc}")
                    for cc in range(NC8)]
            ones2 = None
            if has_bias:
                ones2 = p56.tile([1, TQ], BF16, tag="ones2", name="ones2")
                nc.vector.memset(ones2[:, :], 1.0)

            _ln_transpose(nc, sp, psp, ident, lambda qt: x2_sb[qt][:],
                          xs2T, NQ, wk5)

            # fc1 + gelu
            for ft in range(NF):
                w_t = wp6.tile([128, NC8, 128], BF16, tag="w2_t", name="w2_t",
                               bufs=3)
                nc.sync.dma_start(
                    w_t[:],
                    w2aug[0:C, ts(ft, 128)].rearrange("(cc p) n -> p cc n",
                                                      p=128))
                if has_bias:
                    w_aug = wp6.tile([1, 128], BF16, tag="w2_aug", name="w2_aug")
                    nc.sync.dma_start(w_aug[:], w2aug[C:C + 1, ts(ft, 128)])
                for tb in range(TQ // 512):
                    ps = psa.tile([128, 512], F32, tag="acc", name="ps_f1")
                    for cc in range(NC8):
                        nc.tensor.matmul(ps[:], w_t[:, cc, :],
                                         xs2T[cc][:, ts(tb, 512)],
                                         start=(cc == 0),
                                         stop=(cc == NC8 - 1 and not has_bias))
                    if has_bias:
                        nc.tensor.matmul(ps[:], w_aug[:],
                                         ones2[:, ts(tb, 512)],
                                         start=False, stop=True)
                    nc.scalar.activation(geluT[ft][:, ts(tb, 512)], ps[:], gelu_fn)

        # fc2 + residual
        with tc.tile_pool(name="wk7", bufs=2) as wk7:
            for cb in range(2):
                for qt in range(NQ):
                    ps = psa.tile([128, 512], F32, tag="acc", name="ps_f2")
                    for ft in range(NF):
                        nc.tensor.matmul(ps[:], geluT[ft][:, ts(qt, 128)],
                                         w3_ts[(cb, ft)][:],
                                         start=(ft == 0),
                                         stop=(ft == NF - 1 and not has_bias))
                    if has_bias:
                        nc.tensor.matmul(ps[:], ones_b16[:, ts(qt, 128)],
                                         w3_ts[(cb, "aug")][:],
                                         start=False, stop=True)
                    out_t = wk7.tile([128, 512], F32, tag="out_t", name="out_t")
                    nc.vector.tensor_add(out_t[:], ps[:],
                                         x2_sb[qt][:, ts(cb, 512)])
                    nc.sync.dma_start(out[ts(qt, 128), ts(cb, 512)], out_t[:])


def host_prep(inputs):
    """Build per-core input maps (all numpy, layout/weight-folding only)."""
    x = np.asarray(inputs["hidden_states"], np.float32)
    w_attn = np.asarray(inputs["w_attn"], np.float32)
    b_attn = np.asarray(inputs["b_attn"], np.float32)
    w_proj = np.asarray(inputs["w_proj"], np.float32)
    b_proj = np.asarray(inputs["b_proj"], np.float32)
    ln1_g = np.asarray(inputs["ln1_g"], np.float32)
    ln1_b = np.asarray(inputs["ln1_b"], np.float32)
    ln2_g = np.asarray(inputs["ln2_g"], np.float32)
    ln2_b = np.asarray(inputs["ln2_b"], np.float32)
    w_fc = np.asarray(inputs["w_fc"], np.float32)
    b_fc = np.asarray(inputs["b_fc"], np.float32)
    w_fc2 = np.asarray(inputs["w_fc2"], np.float32)
    b_fc2 = np.asarray(inputs["b_fc2"], np.float32)

    W1 = ln1_g[:, None] * w_attn
    bias1 = ln1_b @ w_attn + b_attn
    w1aug = np.concatenate([W1, bias1[None, :]], 0).astype(ml_dtypes.bfloat16)
    wpaug = np.concatenate([w_proj, b_proj[None, :]], 0).astype(ml_dtypes.bfloat16)
    W2 = ln2_g[:, None] * w_fc
    bias2 = ln2_b @ w_fc + b_fc
    w2aug = np.concatenate([W2, bias2[None, :]], 0).astype(ml_dtypes.bfloat16)
    w3aug = np.concatenate([w_fc2, b_fc2[None, :]], 0).astype(ml_dtypes.bfloat16)
    has_bias = bool(np.any(bias1) or np.any(bias2) or np.any(b_proj)
                    or np.any(b_fc2))

    in_maps = []
    slices = []
    karr = np.arange(T)
    for c in range(NCORES):
        b, j = c // 2, c % 2
        blockA = np.arange(j * 512, (j + 1) * 512)
        blockB = np.arange(1024 + j * 512, 1536 + j * 512)
        own = np.concatenate([blockA, blockB])
        xq_np = np.ascontiguousarray(x[b][own])
        maskc = np.empty((T, 512), np.float32)
        maskc[:1024] = (karr[:1024, None] <= blockA[None, :])
        maskc[1024:] = (karr[1024:, None] <= blockB[None, :])
        in_maps.append({
            "xb": np.ascontiguousarray(x[b]), "xq": xq_np,
            "maskc": maskc.astype(ml_dtypes.bfloat16),
            "w1aug": w1aug, "wpaug": wpaug, "w2aug": w2aug, "w3aug": w3aug,
        })
        slices.append((b, own))
    return in_maps, slices, has_bias


_NC_CACHE = {}


def kernel(**inputs):
    in_maps, slices, has_bias = host_prep(inputs)
    if has_bias not in _NC_CACHE:
        _NC_CACHE[has_bias] = build_program(has_bias=has_bias)
    nc = _NC_CACHE[has_bias]
    res = run_bass_kernel_spmd(nc, in_maps, list(range(NCORES)))
    out = np.empty((B, T, C), np.float32)
    for c, (b, own) in enumerate(slices):
        out[b, own] = res.results[c]["out"]
    return out
